# revision 12
# baseline (speedup 1.0000x reference)
"""Trainium2 Bass kernel for CustomTemporalAttention.

B=8, T=1024, E=1024, H=16, D=64. Pure batch data-parallel across 8 cores.

v3: restructured schedule around dedicated PSUM roles so the PE never
blocks on the softmax pipeline and the HAM clock stays warm:

  - PSUM (8 banks): S ping-pong 2x[128,1024] (4) + projection slot
    1x[128,1024] (2) + PV accumulator 1x[128,1024] (2).
  - PV is head-SERIALIZED per pair: head A ([vA|1], M=65) accumulates
    c0..7 while head B's pt tiles are retained in SBUF; A is evacuated,
    then head B re-uses the same banks as a dense 16-MM burst with
    M=64 at base partition 64 plus a CONCURRENT col-tiled M=1 ones
    matmul at partition 0 for the denominator (HW-verified exact).
  - softmax denominators: one PSUM->SBUF copy carries num+den; den rows
    bounce through DRAM ([2048] -> [128,16]) for a partition-parallel
    reciprocal, broadcast back as stride-0 reads; final normalize is two
    bf16 DVE mults writing oT[0:64]/[64:128] directly (no gpsimd, no
    odd-head bounce).
  - exp-bias slabs are FORWARD-readable: the Toeplitz window DMA uses a
    negative partition stride (HW-verified exact), so the DVE multiply
    reads step +1 / 4B-aligned and packs 2 bf16 per cycle.
  - q-evac on ACT (Copy) with the bias folded in via a K=1 ones-row
    matmul; k-evac on DVE tensor_scalar (engine balance).  Projection
    MMs and their evacuations are emitted at different chunks so the
    strict per-engine FIFOs never stall the exp/mult pipeline.
  - inputs spread over sync+scalar HWDGE queues; weights + table bounce
    on the gpsimd SWDGE queue (first weight tiles queued AHEAD of the
    phase-0 chain); 50 warmup matmuls hold the HAM clock until real
    work lands.
"""

import sys

sys.path.insert(0, "/opt/trn_rl_repo")

import ml_dtypes
import numpy as np

import concourse.bass as bass
import concourse.mybir as mybir
import concourse.tile as tile
from concourse.bass_utils import run_bass_kernel_spmd

F32 = mybir.dt.float32
BF16 = mybir.dt.bfloat16
AF = mybir.ActivationFunctionType
ALU = mybir.AluOpType

B, T, E, H = 8, 1024, 1024, 16
D = E // H  # 64
TQ = 512
W_EB = 1920  # eb slab width
ERB_STRIDE = 2048  # per-head stride of blended-table DRAM rows
# Forward-readable slabs: the table is stored reversed in DRAM and each
# partition's slab line is read with an innermost -1 stride (HW-verified),
# so the hot DVE multiply reads step +1 / 4B-aligned and packs 2 bf16/cycle.
# False falls back to v2-style reversed DVE reads.
EB_FWD = True


def _split_multi_waits(nc):
    """This walrus build accepts at most one sync-wait per instruction; hoist
    extras onto same-engine NoOp carriers placed immediately before."""
    n = 0
    for f in nc.m.functions:
        for blk in f.blocks:
            out = []
            for inst in blk.instructions:
                si = inst.sync_info
                waits = list(si.on_wait) if si and si.on_wait else []
                if len(waits) > 1:
                    for w in waits[:-1]:
                        n += 1
                        nop = mybir.InstNoOp(name=f"{inst.name}-ws{n}", ins=[], outs=[])
                        nop.engine = inst.engine
                        nop.sync_info = mybir.SyncInfo(on_wait=[w], on_update=[])
                        out.append(nop)
                    inst.sync_info = mybir.SyncInfo(
                        on_wait=[waits[-1]], on_update=list(si.on_update or [])
                    )
                out.append(inst)
            blk.instructions = out
    return n


def _craft(ap, dims, offset=None):
    c = ap.copy()
    c.ap = ap.ap.__class__(dims)
    if offset is not None:
        c.offset = offset
    return c


def _build():
    nc = bass.Bass()

    xqT = nc.declare_dram_parameter("xqT", [E, T], BF16, isOutput=False)
    xkT = nc.declare_dram_parameter("xkT", [E, T], BF16, isOutput=False)
    xvT = nc.declare_dram_parameter("xvT", [E, T], BF16, isOutput=False)
    wqT = nc.declare_dram_parameter("wqT", [E, E], BF16, isOutput=False)
    wkT = nc.declare_dram_parameter("wkT", [E, E], BF16, isOutput=False)
    wvT = nc.declare_dram_parameter("wvT", [E, E], BF16, isOutput=False)
    woT = nc.declare_dram_parameter("woT", [E, E], BF16, isOutput=False)
    bqr = nc.declare_dram_parameter("bqr", [E], BF16, isOutput=False)
    bk2 = nc.declare_dram_parameter("bk2", [128, 8], F32, isOutput=False)
    bv1 = nc.declare_dram_parameter("bv1", [E], F32, isOutput=False)
    bo1 = nc.declare_dram_parameter("bo1", [E], F32, isOutput=False)
    rt3 = nc.declare_dram_parameter("rt3", [128, 288], F32, isOutput=False)
    offs = nc.declare_dram_parameter("offs", [1], F32, isOutput=False)
    y_out = nc.declare_dram_parameter("y", [T, E], F32, isOutput=True)

    with tile.TileContext(nc) as tc:
        with (
            tc.tile_pool(name="persist", bufs=1) as persist,
            tc.tile_pool(name="small", bufs=1) as small,
            tc.tile_pool(name="dram", bufs=1, space="DRAM") as drp,
        ):
            # persistent SBUF state
            xq = persist.tile([128, 8, T], BF16, tag="xq")
            xk = persist.tile([128, 8, T], BF16, tag="xk")
            xv = persist.tile([128, 8, T], BF16, tag="xv")
            qT = persist.tile([128, 8, T], BF16, tag="qT")
            kT = persist.tile([128, 8, T], BF16, tag="kT")
            oT = persist.tile([128, 8, T], BF16, tag="oT")
            # per-pair value pages: [vA(0:64) | 1 | vB(65:129) | 1 | pad]
            vp = persist.tile([128, 8, 8, 132], BF16, tag="vp")
            bvrep = persist.tile([128, E], F32, tag="bvrep")
            borep = persist.tile([128, E], F32, tag="borep")
            bks = small.tile([128, 8], F32, tag="bks")
            bqrow = small.tile([1, E], BF16, tag="bqrow")
            ones8 = small.tile([128, 8], BF16, tag="ones8")
            ones_row = small.tile([1, TQ], BF16, tag="ones_row")
            nc.vector.memset(ones8[:], 1.0)
            nc.vector.memset(ones_row[:], 1.0)
            nc.vector.memset(vp[:, :, :, 64:65], 1.0)
            nc.vector.memset(vp[:, :, :, 129:130], 1.0)

            # small loads
            nc.sync.dma_start(out=bks[:], in_=bk2[:])
            nc.sync.dma_start(out=bvrep[:], in_=_craft(bv1[:], [[0, 128], [1, E]], 0))
            nc.sync.dma_start(out=borep[:], in_=_craft(bo1[:], [[0, 128], [1, E]], 0))
            nc.gpsimd.dma_start(out=bqrow[:], in_=bqr[None, :])

            with (
                tc.tile_pool(name="wt8", bufs=3) as wt8p,     # [128,8,128] q/k w chunks
                tc.tile_pool(name="wmv", bufs=1) as wmvp,     # [128,1024] wv then wo
                tc.tile_pool(name="eb", bufs=3) as ebp,       # [128,1920] exp-bias slabs
                tc.tile_pool(name="pt0", bufs=3) as pt0p,     # exp(S) pre-bias
                tc.tile_pool(name="ptA", bufs=3) as ptAp,     # P tiles head A
                tc.tile_pool(name="ptB", bufs=8) as ptBp,     # P tiles head B (full pair)
                tc.tile_pool(name="onum", bufs=2) as onp,     # staged num+den
                tc.tile_pool(name="rp2", bufs=2) as rp2p,     # bcast reciprocals
                tc.tile_pool(name="d16", bufs=2) as d16p,
                tc.tile_pool(name="yst", bufs=2) as ystp,
                tc.tile_pool(name="SP", bufs=2, space="PSUM") as spp,   # S ping-pong
                tc.tile_pool(name="PJ", bufs=1, space="PSUM") as pjp,   # projections
                tc.tile_pool(name="OP", bufs=1, space="PSUM") as opp,   # PV accum
                tc.tile_pool(name="dr2", bufs=2, space="DRAM") as drp2,
            ):
                # ---------- warmup: hold the PE HAM clock at 8/8 ----------
                wk_ps = pjp.tile([128, 2 * TQ], F32, tag="PJ", name="warm")
                for i in range(50):
                    nc.tensor.matmul(
                        wk_ps[0:8, 0:8], ones8[:], ones8[:], start=True, stop=True
                    )

                # ---------- first weight tiles AHEAD of phase-0 on gpsimd ----
                wt8_tiles = {}

                def emit_w_dma(name, w_in, fo):
                    wt8 = wt8p.tile([128, 8, 128], BF16, tag="wt8", name=f"w{name}{fo}")
                    nc.gpsimd.dma_start(
                        out=wt8[:],
                        in_=w_in[:, 128 * fo : 128 * fo + 128].rearrange(
                            "(e p) f -> p e f", p=128
                        ),
                    )
                    wt8_tiles[(name, fo)] = wt8

                emit_w_dma("q", wqT, 0)
                emit_w_dma("q", wqT, 1)
                emit_w_dma("k", wkT, 0)

                # ---------- phase 0: blended exp(bias) table (forward) ------
                p0ctx = tc.tile_pool(name="p0", bufs=1)
                p0 = p0ctx.__enter__()
                rt3s = p0.tile([128, 288], F32, tag="rt3s")
                nc.scalar.dma_start(out=rt3s[:], in_=rt3[:])
                off_sb = p0.tile([1, 1], F32, tag="off")
                nc.scalar.dma_start(out=off_sb[:], in_=offs[None, :])
                th = p0.tile([1, 1], F32, tag="th")
                nc.scalar.activation(th[:], off_sb[:], AF.Tanh)
                w8 = p0.tile([1, 1], F32, tag="w8")
                nc.vector.tensor_scalar_mul(w8[:], th[:], 4.0)  # 8*u = 4*tanh
                abc = p0.tile([1, 3], F32, tag="abc")
                nc.vector.tensor_scalar(abc[:, 0:1], w8[:], -1.0, 0.0, ALU.mult, ALU.max)
                nc.vector.tensor_scalar(abc[:, 2:3], w8[:], 1.0, 0.0, ALU.mult, ALU.max)
                tsum = p0.tile([1, 1], F32, tag="tsum")
                nc.vector.tensor_tensor(tsum[:], abc[:, 0:1], abc[:, 2:3], ALU.add)
                nc.vector.tensor_scalar(abc[:, 1:2], tsum[:], -1.0, 8.0, ALU.mult, ALU.add)
                abc_dram = drp.tile([3], F32, tag="abc_dram")
                nc.gpsimd.dma_start(out=abc_dram[None, :], in_=abc[:])
                abc128 = p0.tile([128, 3], F32, tag="abc128")
                nc.gpsimd.dma_start(
                    out=abc128[:], in_=_craft(abc_dram[:], [[0, 128], [1, 3]], 0)
                )

                # taps: rt3s viewed [128, 16h, 18]; blend 3 shifted slices
                rt3v = rt3s[:].rearrange("p (h i) -> p h i", i=18)
                rb3 = p0.tile([128, 16, 16], F32, tag="rb3")
                tmp3 = p0.tile([128, 16, 16], F32, tag="tmp3")
                a0, a2 = (2, 0)  # reversed-table blend weights (both eb modes)
                nc.vector.tensor_scalar(
                    rb3[:], rt3v[:, :, a0 : a0 + 16], abc128[:, 0:1], None, ALU.mult
                )
                nc.vector.tensor_scalar(
                    tmp3[:], rt3v[:, :, 1:17], abc128[:, 1:2], None, ALU.mult
                )
                nc.vector.tensor_tensor(rb3[:], rb3[:], tmp3[:], ALU.add)
                nc.vector.tensor_scalar(
                    tmp3[:], rt3v[:, :, a2 : a2 + 16], abc128[:, 2:3], None, ALU.mult
                )
                nc.vector.tensor_tensor(rb3[:], rb3[:], tmp3[:], ALU.add)
                erb3 = p0.tile([128, 16, 16], BF16, tag="erb3")
                nc.scalar.activation(erb3[:], rb3[:], AF.Exp, scale=0.125)
                erb_dram = drp.tile([H * ERB_STRIDE], BF16, tag="erb_dram")
                nc.gpsimd.dma_start(
                    out=_craft(
                        erb_dram[None, :], [[16, 128], [ERB_STRIDE, 16], [1, 16]], 0
                    ),
                    in_=erb3[:],
                )
                p0ctx.__exit__(None, None, None)

                # ---------- bulk input loads: spread across both HWDGE queues ----
                for eo in (0, 2, 4, 6):
                    nc.sync.dma_start(out=xq[:, eo, :], in_=xqT[128 * eo : 128 * eo + 128, :])
                for eo in (1, 3, 5, 7):
                    nc.scalar.dma_start(out=xq[:, eo, :], in_=xqT[128 * eo : 128 * eo + 128, :])
                for eo in (0, 2, 4, 6):
                    nc.sync.dma_start(out=xv[:, eo, :], in_=xvT[128 * eo : 128 * eo + 128, :])
                for eo in (1, 3, 5, 7):
                    nc.scalar.dma_start(out=xv[:, eo, :], in_=xvT[128 * eo : 128 * eo + 128, :])
                wv_tiles = {}
                for eo in range(8):
                    wt_ = wmvp.tile([128, 2 * TQ], BF16, tag=f"wmv{eo}", name=f"wv{eo}")
                    q = nc.sync if eo % 2 == 0 else nc.scalar
                    q.dma_start(out=wt_[:], in_=wvT[128 * eo : 128 * eo + 128, :])
                    wv_tiles[eo] = wt_
                for eo in (0, 2, 4, 6):
                    nc.sync.dma_start(out=xk[:, eo, :], in_=xkT[128 * eo : 128 * eo + 128, :])
                for eo in (1, 3, 5, 7):
                    nc.scalar.dma_start(out=xk[:, eo, :], in_=xkT[128 * eo : 128 * eo + 128, :])

                # ---------- projection jobs (MMs and evacs split) ----------
                pj_state = {}

                def emit_q_mms(fo, tag):
                    wt8 = wt8_tiles.pop(("q", fo))
                    pool = spp if tag == "S" else pjp
                    sp = pool.tile([128, 2 * TQ], F32, tag=tag, name=f"pq{fo}")
                    for tqh in range(2):
                        nc.tensor.matmul(
                            sp[:, TQ * tqh : TQ * tqh + TQ],
                            bqrow[0:1, 128 * fo : 128 * fo + 128],
                            ones_row[0:1, :],
                            start=True,
                            stop=False,
                        )
                        for eo in range(8):
                            nc.tensor.matmul(
                                sp[:, TQ * tqh : TQ * tqh + TQ],
                                wt8[:, eo, :],
                                xq[:, eo, TQ * tqh : TQ * tqh + TQ],
                                start=False,
                                stop=(eo == 7),
                            )
                    pj_state[("q", fo)] = sp

                def emit_q_evac(fo):
                    sp = pj_state.pop(("q", fo))
                    nc.scalar.activation(qT[:, fo, :], sp[:], AF.Copy)

                def emit_k_mms(fo, tag):
                    wt8 = wt8_tiles.pop(("k", fo))
                    pool = spp if tag == "S" else pjp
                    sp = pool.tile([128, 2 * TQ], F32, tag=tag, name=f"pk{fo}")
                    for tqh in range(2):
                        for eo in range(8):
                            nc.tensor.matmul(
                                sp[:, TQ * tqh : TQ * tqh + TQ],
                                wt8[:, eo, :],
                                xk[:, eo, TQ * tqh : TQ * tqh + TQ],
                                start=(eo == 0),
                                stop=(eo == 7),
                            )
                    pj_state[("k", fo)] = sp

                def emit_k_evac(fo):
                    sp = pj_state.pop(("k", fo))
                    nc.vector.tensor_scalar(
                        kT[:, fo, :], sp[:], 1.0, bks[:, fo : fo + 1], ALU.mult, ALU.add
                    )

                def emit_v_job(to, tag):
                    pool = spp if tag == "S" else pjp
                    sp = pool.tile([128, 2 * TQ], F32, tag=tag, name=f"pv{to}")
                    for fv in range(2):
                        for eo in range(8):
                            nc.tensor.matmul(
                                sp[:, TQ * fv : TQ * fv + TQ],
                                xv[:, eo, 128 * to : 128 * to + 128],
                                wv_tiles[eo][:, TQ * fv : TQ * fv + TQ],
                                start=(eo == 0),
                                stop=(eo == 7),
                            )
                    # scatter into [vA |1| vB |1| pad] pages (+ bias)
                    spv = sp[:].rearrange("p (pr x) -> p pr x", x=128)
                    bvv = bvrep[:].rearrange("p (pr x) -> p pr x", x=128)
                    for hi in range(2):
                        nc.vector.tensor_tensor(
                            vp[:, to, :, 65 * hi : 65 * hi + 64],
                            spv[:, :, 64 * hi : 64 * hi + 64],
                            bvv[:, :, 64 * hi : 64 * hi + 64],
                            ALU.add,
                        )

                wo_tiles = {}

                def emit_wo_dma(co):
                    wt_ = wmvp.tile([128, 2 * TQ], BF16, tag=f"wmv{co}", name=f"wo{co}")
                    nc.scalar.dma_start(out=wt_[:], in_=woT[128 * co : 128 * co + 128, :])
                    wo_tiles[co] = wt_

                # ---------- eb slab prefetch (forward via negative stride) ----
                ebs = {}

                def emit_eb(hh):
                    eb_ = ebp.tile([128, W_EB], BF16, tag="eb", name=f"eb{hh}")
                    q = nc.sync if hh % 2 == 0 else nc.scalar
                    if EB_FWD:
                        src = _craft(
                            erb_dram[None, :],
                            [[1, 128], [-1, W_EB]],
                            hh * ERB_STRIDE + 1919,
                        )
                    else:
                        src = _craft(
                            erb_dram[None, :], [[1, 128], [1, W_EB]], hh * ERB_STRIDE
                        )
                    q.dma_start(out=eb_[:], in_=src)
                    ebs[hh] = eb_

                # ---------- normalization chain ----------
                norm_state = {}

                def emit_stage_evacA(p, opA):
                    onA = onp.tile([128, 2 * TQ], BF16, tag="onum", name=f"onA{p}")
                    nc.vector.tensor_copy(out=onA[0:65, :], in_=opA[0:65, :])
                    dd = drp2.tile([2 * 2 * TQ], BF16, tag="dend", name=f"dd{p}")
                    nc.gpsimd.dma_start(
                        out=_craft(dd[None, :], [[0, 1], [1, 2 * TQ]], 0),
                        in_=onA[64:65, :],
                    )
                    norm_state[p] = {"onA": onA, "dd": dd}

                def emit_stage_evacB(p):
                    st = norm_state[p]
                    opB = st.pop("opB")
                    onB = onp.tile([128, 2 * TQ], BF16, tag="onum", name=f"onB{p}")
                    nc.vector.tensor_copy(out=onB[64:128, :], in_=opB[64:128, :])
                    nc.scalar.activation(onB[0:1, :], opB[0:1, :], AF.Copy)
                    nc.gpsimd.dma_start(
                        out=_craft(st["dd"][None, :], [[0, 1], [1, 2 * TQ]], 2 * TQ),
                        in_=onB[0:1, :],
                    )
                    st["onB"] = onB

                def emit_norm_d16(p):
                    st = norm_state[p]
                    d16 = d16p.tile([128, 16], BF16, tag="d16", name=f"d16_{p}")
                    nc.gpsimd.dma_start(
                        out=d16[:],
                        in_=_craft(st["dd"][None, :], [[1, 128], [1024, 2], [128, 8]], 0),
                    )
                    st["d16"] = d16

                def emit_norm_recip(p):
                    st = norm_state[p]
                    r16 = d16p.tile([128, 16], BF16, tag="r16", name=f"r16_{p}")
                    with nc.allow_low_precision(reason="bf16 softmax denom ~0.4% ok"):
                        nc.vector.reciprocal(r16[:], st["d16"][:])
                    st["r16"] = r16

                def emit_norm_rdw(p):
                    st = norm_state[p]
                    rd = drp2.tile([2 * 2 * TQ], BF16, tag="recd", name=f"rd{p}")
                    nc.gpsimd.dma_start(
                        out=_craft(rd[None, :], [[1, 128], [1024, 2], [128, 8]], 0),
                        in_=st["r16"][:],
                    )
                    st["rd"] = rd

                def emit_norm_rp2(p):
                    st = norm_state[p]
                    rp2 = rp2p.tile([128, 2 * TQ], BF16, tag="rp2", name=f"rp2_{p}")
                    nc.gpsimd.dma_start(
                        out=rp2[0:64, :],
                        in_=_craft(st["rd"][None, :], [[0, 64], [1, 2 * TQ]], 0),
                    )
                    nc.gpsimd.dma_start(
                        out=rp2[64:128, :],
                        in_=_craft(st["rd"][None, :], [[0, 64], [1, 2 * TQ]], 2 * TQ),
                    )
                    st["rp2"] = rp2

                def emit_norm_final(p):
                    st = norm_state.pop(p)
                    nc.vector.tensor_tensor(
                        oT[0:64, p, :], st["onA"][0:64, :], st["rp2"][0:64, :], ALU.mult
                    )
                    nc.vector.tensor_tensor(
                        oT[64:128, p, :], st["onB"][64:128, :], st["rp2"][64:128, :],
                        ALU.mult,
                    )

                # ---------- attention pair ----------
                def emit_pair(p, hooks):
                    hA, hB = 2 * p, 2 * p + 1
                    ebA, ebB = ebs.pop(hA), ebs.pop(hB)
                    opA = opp.tile([128, 2 * TQ], F32, tag="OP", name=f"opA{p}")
                    ptAs = {}
                    ptBs = {}

                    def emit_pv_a(c):
                        pt_ = ptAs.pop(c)
                        for tqh in range(2):
                            nc.tensor.matmul(
                                opA[0:65, TQ * tqh : TQ * tqh + TQ],
                                vp[:, c, p, 0:65],
                                pt_[:, TQ * tqh : TQ * tqh + TQ],
                                start=(c == 0),
                                stop=(c == 7),
                            )

                    for c in range(8):
                        sps = []
                        for hp0 in (0, 64):
                            sp = spp.tile([128, 2 * TQ], F32, tag="S",
                                          name=f"s{2 * p + hp0 // 64}_{c}")
                            for tqh in range(2):
                                nc.tensor.matmul(
                                    sp[:, TQ * tqh : TQ * tqh + TQ],
                                    kT[hp0 : hp0 + 64, p, 128 * c : 128 * c + 128],
                                    qT[hp0 : hp0 + 64, p, TQ * tqh : TQ * tqh + TQ],
                                    start=True,
                                    stop=True,
                                )
                            sps.append(sp)
                        for fn in hooks.get(c, ()):
                            fn()
                        if EB_FWD:
                            sc = 896 - 128 * c
                        else:
                            s0 = 1023 + 128 * c
                        for hi, (sp, eb_) in enumerate(zip(sps, (ebA, ebB))):
                            pt0 = pt0p.tile([128, 2 * TQ], BF16, tag="pt0")
                            nc.scalar.activation(pt0[:], sp[:], AF.Exp, scale=0.125)
                            ptp_ = ptAp if hi == 0 else ptBp
                            pt_ = ptp_.tile([128, 2 * TQ], BF16,
                                            tag="ptA" if hi == 0 else "ptB",
                                            name=f"pt{2 * p + hi}_{c}")
                            ebv = (
                                eb_[:, sc : sc + 2 * TQ]
                                if EB_FWD
                                else eb_[:, s0 - (2 * TQ - 1) : s0 + 1][:, ::-1]
                            )
                            nc.vector.tensor_tensor(pt_[:], pt0[:], ebv, ALU.mult)
                            (ptAs if hi == 0 else ptBs)[c] = pt_
                        if c >= 1:
                            emit_pv_a(c - 1)
                    emit_pv_a(7)
                    emit_stage_evacA(p, opA)
                    # head B: dense burst into the same banks (M=64 @ base 64
                    # + concurrent col-tiled M=1 denominator @ partition 0)
                    opB = opp.tile([128, 2 * TQ], F32, tag="OP", name=f"opB{p}")
                    for c in range(8):
                        pt_ = ptBs.pop(c)
                        for tqh in range(2):
                            nc.tensor.matmul(
                                opB[64:128, TQ * tqh : TQ * tqh + TQ],
                                vp[:, c, p, 65:129],
                                pt_[:, TQ * tqh : TQ * tqh + TQ],
                                start=(c == 0),
                                stop=(c == 7),
                            )
                            nc.tensor.matmul(
                                opB[0:1, TQ * tqh : TQ * tqh + TQ],
                                ones8[:, 0:1],
                                pt_[:, TQ * tqh : TQ * tqh + TQ],
                                start=(c == 0),
                                stop=(c == 7),
                            )
                    norm_state[p]["opB"] = opB

                # ---------- schedule ----------
                def add_hook(hooks, c, fn):
                    hooks.setdefault(c, []).append(fn)

                # pre-attention ramp: q0,q1 as soon as xq lands; v0 after xv;
                # k0,k1 after xk.  S-tag tiles are free until pair 0.
                emit_q_mms(0, "S")
                emit_q_evac(0)
                emit_q_mms(1, "S")
                emit_q_evac(1)
                emit_v_job(0, "PJ")
                emit_w_dma("k", wkT, 1)
                emit_k_mms(0, "S")
                emit_k_evac(0)
                emit_k_mms(1, "S")
                emit_k_evac(1)
                for hh in range(4):  # eb slabs for pairs 0 and 1
                    emit_eb(hh)

                for p in range(8):
                    hooks = {}
                    if p + 1 < 8:
                        add_hook(hooks, 0, lambda p=p: emit_eb(2 * p + 2))
                        add_hook(hooks, 1, lambda p=p: emit_eb(2 * p + 3))
                    if p == 0:
                        for c, to in ((0, 1), (1, 2), (2, 3), (3, 4), (4, 5), (5, 6), (6, 7)):
                            add_hook(hooks, c, lambda to=to: emit_v_job(to, "PJ"))
                        add_hook(hooks, 2, lambda: emit_w_dma("q", wqT, 2))
                        add_hook(hooks, 5, lambda: emit_w_dma("k", wkT, 2))
                    else:
                        # norm chain for pair p-1 (B-side evac deferred here so
                        # the PV_B burst never stalls this pair's exp/mult)
                        add_hook(hooks, 1, lambda p=p: emit_stage_evacB(p - 1))
                        add_hook(hooks, 2, lambda p=p: emit_norm_d16(p - 1))
                        add_hook(hooks, 3, lambda p=p: emit_norm_recip(p - 1))
                        add_hook(hooks, 4, lambda p=p: emit_norm_rdw(p - 1))
                        add_hook(hooks, 5, lambda p=p: emit_norm_rp2(p - 1))
                        add_hook(hooks, 7, lambda p=p: emit_norm_final(p - 1))
                        # next q/k jobs: MMs and evacs at different chunks
                        if p + 1 < 8:
                            add_hook(hooks, 0, lambda p=p: emit_q_mms(p + 1, "PJ"))
                            add_hook(hooks, 3, lambda p=p: emit_q_evac(p + 1))
                            add_hook(hooks, 4, lambda p=p: emit_k_mms(p + 1, "PJ"))
                            add_hook(hooks, 7, lambda p=p: emit_k_evac(p + 1))
                        if p + 2 < 8:
                            add_hook(hooks, 2, lambda p=p: emit_w_dma("q", wqT, p + 2))
                            add_hook(hooks, 5, lambda p=p: emit_w_dma("k", wkT, p + 2))
                        if 1 <= p <= 4:
                            add_hook(hooks, 3, lambda p=p: emit_wo_dma(2 * p - 2))
                            add_hook(hooks, 6, lambda p=p: emit_wo_dma(2 * p - 1))
                    emit_pair(p, hooks)

                # ---------- tail: norm(7) + output projection ----------
                emit_stage_evacB(7)
                emit_norm_d16(7)
                emit_norm_recip(7)
                emit_norm_rdw(7)
                emit_norm_rp2(7)
                emit_norm_final(7)
                for to in range(8):
                    tag = "PJ" if to % 3 == 2 else "S"
                    pool = pjp if tag == "PJ" else spp
                    sp = pool.tile([128, 2 * TQ], F32, tag=tag, name=f"y{to}")
                    for fh in range(2):
                        for co in range(8):
                            nc.tensor.matmul(
                                sp[:, TQ * fh : TQ * fh + TQ],
                                oT[:, co, 128 * to : 128 * to + 128],
                                wo_tiles[co][:, TQ * fh : TQ * fh + TQ],
                                start=(co == 0),
                                stop=(co == 7),
                            )
                    yst = ystp.tile([128, 2 * TQ], F32, tag="yst")
                    nc.vector.tensor_tensor(yst[:], sp[:], borep[:], ALU.add)
                    q = nc.sync if to % 2 == 0 else nc.scalar
                    q.dma_start(out=y_out[128 * to : 128 * to + 128, :], in_=yst[:])

    _split_multi_waits(nc)
    return nc


_NC_CACHE = None


def _get_nc():
    global _NC_CACHE
    if _NC_CACHE is None:
        _NC_CACHE = _build()
    return _NC_CACHE


def _bf(x):
    return np.ascontiguousarray(np.asarray(x, np.float32).astype(ml_dtypes.bfloat16))


def _prepare_in_maps(
    query, key_, value, Wq, bq, Wk, bk, Wv, bv, Wo, bo, bias_table, offset
):
    query = np.asarray(query, np.float32)
    key_ = np.asarray(key_, np.float32)
    value = np.asarray(value, np.float32)
    shared = {
        "wqT": _bf(np.asarray(Wq, np.float32).T),
        "wkT": _bf(np.asarray(Wk, np.float32).T),
        "wvT": _bf(np.asarray(Wv, np.float32).T),
        "woT": _bf(np.asarray(Wo, np.float32).T),
        "bqr": _bf(np.asarray(bq, np.float32)),
        "bk2": np.ascontiguousarray(np.asarray(bk, np.float32).reshape(8, 128).T),
        "bv1": np.ascontiguousarray(np.asarray(bv, np.float32)),
        "bo1": np.ascontiguousarray(np.asarray(bo, np.float32)),
        "offs": np.ascontiguousarray(np.asarray(offset, np.float32)),
    }
    tab = np.asarray(bias_table, np.float32)  # [2T-1, H]
    padf = np.concatenate([tab[0:1], tab, tab[-1:]], axis=0)  # [2T+1, H] forward
    padfT = padf[::-1].T  # [H, 2T+1] reversed (both eb modes)
    # taps for the 128-partition blend: rt3[p, h*18 + i] = padfT[h, min(16p+i, 2T)]
    idx = np.minimum(np.arange(128)[:, None] * 16 + np.arange(18)[None, :], 2 * T)
    rt3 = padfT[:, idx]  # [H, 128, 18]
    rt3 = np.ascontiguousarray(rt3.transpose(1, 0, 2).reshape(128, 288))
    shared["rt3"] = rt3.astype(np.float32)

    in_maps = []
    for b in range(B):
        m = dict(shared)
        m["xqT"] = _bf(query[b].T)
        m["xkT"] = _bf(key_[b].T)
        m["xvT"] = _bf(value[b].T)
        in_maps.append(m)
    return in_maps


def kernel(**inputs):
    in_maps = _prepare_in_maps(
        inputs["query"], inputs["key_"], inputs["value"],
        inputs["Wq"], inputs["bq"], inputs["Wk"], inputs["bk"],
        inputs["Wv"], inputs["bv"], inputs["Wo"], inputs["bo"],
        inputs["bias_table"], inputs["offset"],
    )
    nc = _get_nc()
    res = run_bass_kernel_spmd(nc, in_maps, list(range(B)))
    out = np.stack([res.results[b]["y"] for b in range(B)], axis=0)
    return out.astype(np.float32)


# revision 13
# speedup vs baseline: 9.5174x; 9.5174x over previous
"""Trainium2 Bass kernel for CustomTemporalAttention.

B=8, T=1024, E=1024, H=16, D=64. Pure batch data-parallel across 8 cores.

v3: restructured schedule around dedicated PSUM roles so the PE never
blocks on the softmax pipeline and the HAM clock stays warm:

  - PSUM (8 banks): S ping-pong 2x[128,1024] (4) + projection slot
    1x[128,1024] (2) + PV accumulator 1x[128,1024] (2).
  - PV is head-SERIALIZED per pair: head A ([vA|1], M=65) accumulates
    c0..7 while head B's pt tiles are retained in SBUF; A is evacuated,
    then head B re-uses the same banks as a dense 16-MM burst with
    M=64 at base partition 64 plus a CONCURRENT col-tiled M=1 ones
    matmul at partition 0 for the denominator (HW-verified exact).
  - softmax denominators: one PSUM->SBUF copy carries num+den; den rows
    bounce through DRAM ([2048] -> [128,16]) for a partition-parallel
    reciprocal, broadcast back as stride-0 reads; final normalize is two
    bf16 DVE mults writing oT[0:64]/[64:128] directly (no gpsimd, no
    odd-head bounce).
  - exp-bias slabs are FORWARD-readable: the Toeplitz window DMA uses a
    negative partition stride (HW-verified exact), so the DVE multiply
    reads step +1 / 4B-aligned and packs 2 bf16 per cycle.
  - q-evac on ACT (Copy) with the bias folded in via a K=1 ones-row
    matmul; k-evac on DVE tensor_scalar (engine balance).  Projection
    MMs and their evacuations are emitted at different chunks so the
    strict per-engine FIFOs never stall the exp/mult pipeline.
  - inputs spread over sync+scalar HWDGE queues; weights + table bounce
    on the gpsimd SWDGE queue (first weight tiles queued AHEAD of the
    phase-0 chain); 50 warmup matmuls hold the HAM clock until real
    work lands.
"""

import sys

sys.path.insert(0, "/opt/trn_rl_repo")

import ml_dtypes
import numpy as np

import concourse.bass as bass
import concourse.mybir as mybir
import concourse.tile as tile
from concourse.bass_utils import run_bass_kernel_spmd

F32 = mybir.dt.float32
BF16 = mybir.dt.bfloat16
AF = mybir.ActivationFunctionType
ALU = mybir.AluOpType

B, T, E, H = 8, 1024, 1024, 16
D = E // H  # 64
TQ = 512
W_EB = 1920  # eb slab width
ERB_STRIDE = 2048  # per-head stride of blended-table DRAM rows
# Forward-readable slabs via innermost -1 stride DMA are functionally
# correct but the DMA degenerates to element-granularity descriptors
# (~17.7us engine time per slab, 11x kernel slowdown) - keep reversed
# DVE reads instead.
EB_FWD = False


def _split_multi_waits(nc):
    """This walrus build accepts at most one sync-wait per instruction; hoist
    extras onto same-engine NoOp carriers placed immediately before."""
    n = 0
    for f in nc.m.functions:
        for blk in f.blocks:
            out = []
            for inst in blk.instructions:
                si = inst.sync_info
                waits = list(si.on_wait) if si and si.on_wait else []
                if len(waits) > 1:
                    for w in waits[:-1]:
                        n += 1
                        nop = mybir.InstNoOp(name=f"{inst.name}-ws{n}", ins=[], outs=[])
                        nop.engine = inst.engine
                        nop.sync_info = mybir.SyncInfo(on_wait=[w], on_update=[])
                        out.append(nop)
                    inst.sync_info = mybir.SyncInfo(
                        on_wait=[waits[-1]], on_update=list(si.on_update or [])
                    )
                out.append(inst)
            blk.instructions = out
    return n


def _craft(ap, dims, offset=None):
    c = ap.copy()
    c.ap = ap.ap.__class__(dims)
    if offset is not None:
        c.offset = offset
    return c


def _build():
    nc = bass.Bass()

    xqT = nc.declare_dram_parameter("xqT", [E, T], BF16, isOutput=False)
    xkT = nc.declare_dram_parameter("xkT", [E, T], BF16, isOutput=False)
    xvT = nc.declare_dram_parameter("xvT", [E, T], BF16, isOutput=False)
    wqT = nc.declare_dram_parameter("wqT", [E, E], BF16, isOutput=False)
    wkT = nc.declare_dram_parameter("wkT", [E, E], BF16, isOutput=False)
    wvT = nc.declare_dram_parameter("wvT", [E, E], BF16, isOutput=False)
    woT = nc.declare_dram_parameter("woT", [E, E], BF16, isOutput=False)
    bqr = nc.declare_dram_parameter("bqr", [E], BF16, isOutput=False)
    bk2 = nc.declare_dram_parameter("bk2", [128, 8], F32, isOutput=False)
    bv1 = nc.declare_dram_parameter("bv1", [E], F32, isOutput=False)
    bo1 = nc.declare_dram_parameter("bo1", [E], F32, isOutput=False)
    rt3 = nc.declare_dram_parameter("rt3", [128, 288], F32, isOutput=False)
    offs = nc.declare_dram_parameter("offs", [1], F32, isOutput=False)
    y_out = nc.declare_dram_parameter("y", [T, E], F32, isOutput=True)

    with tile.TileContext(nc) as tc:
        with (
            tc.tile_pool(name="persist", bufs=1) as persist,
            tc.tile_pool(name="small", bufs=1) as small,
            tc.tile_pool(name="dram", bufs=1, space="DRAM") as drp,
        ):
            # persistent SBUF state
            xq = persist.tile([128, 8, T], BF16, tag="xq")
            xk = persist.tile([128, 8, T], BF16, tag="xk")
            xv = persist.tile([128, 8, T], BF16, tag="xv")
            qT = persist.tile([128, 8, T], BF16, tag="qT")
            kT = persist.tile([128, 8, T], BF16, tag="kT")
            oT = persist.tile([128, 8, T], BF16, tag="oT")
            # per-pair value pages: [vA(0:64) | 1 | vB(65:129) | 1 | pad]
            vp = persist.tile([128, 8, 8, 132], BF16, tag="vp")
            bvrep = persist.tile([128, E], F32, tag="bvrep")
            borep = persist.tile([128, E], F32, tag="borep")
            bks = small.tile([128, 8], F32, tag="bks")
            bqrow = small.tile([1, E], BF16, tag="bqrow")
            ones8 = small.tile([128, 8], BF16, tag="ones8")
            ones_row = small.tile([1, TQ], BF16, tag="ones_row")
            nc.vector.memset(ones8[:], 1.0)
            nc.vector.memset(ones_row[:], 1.0)
            nc.vector.memset(vp[:, :, :, 64:65], 1.0)
            nc.vector.memset(vp[:, :, :, 129:130], 1.0)

            # small loads
            nc.sync.dma_start(out=bks[:], in_=bk2[:])
            nc.sync.dma_start(out=bvrep[:], in_=_craft(bv1[:], [[0, 128], [1, E]], 0))
            nc.sync.dma_start(out=borep[:], in_=_craft(bo1[:], [[0, 128], [1, E]], 0))
            nc.gpsimd.dma_start(out=bqrow[:], in_=bqr[None, :])

            with (
                tc.tile_pool(name="wt8", bufs=3) as wt8p,     # [128,8,128] q/k w chunks
                tc.tile_pool(name="wmv", bufs=1) as wmvp,     # [128,1024] wv then wo
                tc.tile_pool(name="eb", bufs=3) as ebp,       # [128,1920] exp-bias slabs
                tc.tile_pool(name="pt0", bufs=3) as pt0p,     # exp(S) pre-bias
                tc.tile_pool(name="ptA", bufs=3) as ptAp,     # P tiles head A
                tc.tile_pool(name="ptB", bufs=8) as ptBp,     # P tiles head B (full pair)
                tc.tile_pool(name="onum", bufs=2) as onp,     # staged num+den
                tc.tile_pool(name="rp2", bufs=2) as rp2p,     # bcast reciprocals
                tc.tile_pool(name="d16", bufs=2) as d16p,
                tc.tile_pool(name="yst", bufs=2) as ystp,
                tc.tile_pool(name="SP", bufs=2, space="PSUM") as spp,   # S ping-pong
                tc.tile_pool(name="PJ", bufs=1, space="PSUM") as pjp,   # projections
                tc.tile_pool(name="OP", bufs=1, space="PSUM") as opp,   # PV accum
                tc.tile_pool(name="dr2", bufs=2, space="DRAM") as drp2,
            ):
                # ---------- warmup: hold the PE HAM clock at 8/8 ----------
                wk_ps = pjp.tile([128, 2 * TQ], F32, tag="PJ", name="warm")
                for i in range(50):
                    nc.tensor.matmul(
                        wk_ps[0:8, 0:8], ones8[:], ones8[:], start=True, stop=True
                    )

                # ---------- first weight tiles AHEAD of phase-0 on gpsimd ----
                wt8_tiles = {}

                def emit_w_dma(name, w_in, fo):
                    wt8 = wt8p.tile([128, 8, 128], BF16, tag="wt8", name=f"w{name}{fo}")
                    nc.gpsimd.dma_start(
                        out=wt8[:],
                        in_=w_in[:, 128 * fo : 128 * fo + 128].rearrange(
                            "(e p) f -> p e f", p=128
                        ),
                    )
                    wt8_tiles[(name, fo)] = wt8

                emit_w_dma("q", wqT, 0)
                emit_w_dma("q", wqT, 1)
                emit_w_dma("k", wkT, 0)

                # ---------- phase 0: blended exp(bias) table (forward) ------
                p0ctx = tc.tile_pool(name="p0", bufs=1)
                p0 = p0ctx.__enter__()
                rt3s = p0.tile([128, 288], F32, tag="rt3s")
                nc.scalar.dma_start(out=rt3s[:], in_=rt3[:])
                off_sb = p0.tile([1, 1], F32, tag="off")
                nc.scalar.dma_start(out=off_sb[:], in_=offs[None, :])
                th = p0.tile([1, 1], F32, tag="th")
                nc.scalar.activation(th[:], off_sb[:], AF.Tanh)
                w8 = p0.tile([1, 1], F32, tag="w8")
                nc.vector.tensor_scalar_mul(w8[:], th[:], 4.0)  # 8*u = 4*tanh
                abc = p0.tile([1, 3], F32, tag="abc")
                nc.vector.tensor_scalar(abc[:, 0:1], w8[:], -1.0, 0.0, ALU.mult, ALU.max)
                nc.vector.tensor_scalar(abc[:, 2:3], w8[:], 1.0, 0.0, ALU.mult, ALU.max)
                tsum = p0.tile([1, 1], F32, tag="tsum")
                nc.vector.tensor_tensor(tsum[:], abc[:, 0:1], abc[:, 2:3], ALU.add)
                nc.vector.tensor_scalar(abc[:, 1:2], tsum[:], -1.0, 8.0, ALU.mult, ALU.add)
                abc_dram = drp.tile([3], F32, tag="abc_dram")
                nc.gpsimd.dma_start(out=abc_dram[None, :], in_=abc[:])
                abc128 = p0.tile([128, 3], F32, tag="abc128")
                nc.gpsimd.dma_start(
                    out=abc128[:], in_=_craft(abc_dram[:], [[0, 128], [1, 3]], 0)
                )

                # taps: rt3s viewed [128, 16h, 18]; blend 3 shifted slices
                rt3v = rt3s[:].rearrange("p (h i) -> p h i", i=18)
                rb3 = p0.tile([128, 16, 16], F32, tag="rb3")
                tmp3 = p0.tile([128, 16, 16], F32, tag="tmp3")
                a0, a2 = (2, 0)  # reversed-table blend weights (both eb modes)
                nc.vector.tensor_scalar(
                    rb3[:], rt3v[:, :, a0 : a0 + 16], abc128[:, 0:1], None, ALU.mult
                )
                nc.vector.tensor_scalar(
                    tmp3[:], rt3v[:, :, 1:17], abc128[:, 1:2], None, ALU.mult
                )
                nc.vector.tensor_tensor(rb3[:], rb3[:], tmp3[:], ALU.add)
                nc.vector.tensor_scalar(
                    tmp3[:], rt3v[:, :, a2 : a2 + 16], abc128[:, 2:3], None, ALU.mult
                )
                nc.vector.tensor_tensor(rb3[:], rb3[:], tmp3[:], ALU.add)
                erb3 = p0.tile([128, 16, 16], BF16, tag="erb3")
                nc.scalar.activation(erb3[:], rb3[:], AF.Exp, scale=0.125)
                erb_dram = drp.tile([H * ERB_STRIDE], BF16, tag="erb_dram")
                nc.gpsimd.dma_start(
                    out=_craft(
                        erb_dram[None, :], [[16, 128], [ERB_STRIDE, 16], [1, 16]], 0
                    ),
                    in_=erb3[:],
                )
                p0ctx.__exit__(None, None, None)

                # ---------- bulk input loads: spread across both HWDGE queues ----
                for eo in (0, 2, 4, 6):
                    nc.sync.dma_start(out=xq[:, eo, :], in_=xqT[128 * eo : 128 * eo + 128, :])
                for eo in (1, 3, 5, 7):
                    nc.scalar.dma_start(out=xq[:, eo, :], in_=xqT[128 * eo : 128 * eo + 128, :])
                for eo in (0, 2, 4, 6):
                    nc.sync.dma_start(out=xv[:, eo, :], in_=xvT[128 * eo : 128 * eo + 128, :])
                for eo in (1, 3, 5, 7):
                    nc.scalar.dma_start(out=xv[:, eo, :], in_=xvT[128 * eo : 128 * eo + 128, :])
                wv_tiles = {}
                for eo in range(8):
                    wt_ = wmvp.tile([128, 2 * TQ], BF16, tag=f"wmv{eo}", name=f"wv{eo}")
                    q = nc.sync if eo % 2 == 0 else nc.scalar
                    q.dma_start(out=wt_[:], in_=wvT[128 * eo : 128 * eo + 128, :])
                    wv_tiles[eo] = wt_
                for eo in (0, 2, 4, 6):
                    nc.sync.dma_start(out=xk[:, eo, :], in_=xkT[128 * eo : 128 * eo + 128, :])
                for eo in (1, 3, 5, 7):
                    nc.scalar.dma_start(out=xk[:, eo, :], in_=xkT[128 * eo : 128 * eo + 128, :])

                # ---------- projection jobs (MMs and evacs split) ----------
                pj_state = {}

                def emit_q_mms(fo, tag):
                    wt8 = wt8_tiles.pop(("q", fo))
                    pool = spp if tag == "S" else pjp
                    sp = pool.tile([128, 2 * TQ], F32, tag=tag, name=f"pq{fo}")
                    for tqh in range(2):
                        nc.tensor.matmul(
                            sp[:, TQ * tqh : TQ * tqh + TQ],
                            bqrow[0:1, 128 * fo : 128 * fo + 128],
                            ones_row[0:1, :],
                            start=True,
                            stop=False,
                        )
                        for eo in range(8):
                            nc.tensor.matmul(
                                sp[:, TQ * tqh : TQ * tqh + TQ],
                                wt8[:, eo, :],
                                xq[:, eo, TQ * tqh : TQ * tqh + TQ],
                                start=False,
                                stop=(eo == 7),
                            )
                    pj_state[("q", fo)] = sp

                def emit_q_evac(fo):
                    sp = pj_state.pop(("q", fo))
                    nc.scalar.activation(qT[:, fo, :], sp[:], AF.Copy)

                def emit_k_mms(fo, tag):
                    wt8 = wt8_tiles.pop(("k", fo))
                    pool = spp if tag == "S" else pjp
                    sp = pool.tile([128, 2 * TQ], F32, tag=tag, name=f"pk{fo}")
                    for tqh in range(2):
                        for eo in range(8):
                            nc.tensor.matmul(
                                sp[:, TQ * tqh : TQ * tqh + TQ],
                                wt8[:, eo, :],
                                xk[:, eo, TQ * tqh : TQ * tqh + TQ],
                                start=(eo == 0),
                                stop=(eo == 7),
                            )
                    pj_state[("k", fo)] = sp

                def emit_k_evac(fo):
                    sp = pj_state.pop(("k", fo))
                    nc.vector.tensor_scalar(
                        kT[:, fo, :], sp[:], 1.0, bks[:, fo : fo + 1], ALU.mult, ALU.add
                    )

                def emit_v_job(to, tag):
                    pool = spp if tag == "S" else pjp
                    sp = pool.tile([128, 2 * TQ], F32, tag=tag, name=f"pv{to}")
                    for fv in range(2):
                        for eo in range(8):
                            nc.tensor.matmul(
                                sp[:, TQ * fv : TQ * fv + TQ],
                                xv[:, eo, 128 * to : 128 * to + 128],
                                wv_tiles[eo][:, TQ * fv : TQ * fv + TQ],
                                start=(eo == 0),
                                stop=(eo == 7),
                            )
                    # scatter into [vA |1| vB |1| pad] pages (+ bias)
                    spv = sp[:].rearrange("p (pr x) -> p pr x", x=128)
                    bvv = bvrep[:].rearrange("p (pr x) -> p pr x", x=128)
                    for hi in range(2):
                        nc.vector.tensor_tensor(
                            vp[:, to, :, 65 * hi : 65 * hi + 64],
                            spv[:, :, 64 * hi : 64 * hi + 64],
                            bvv[:, :, 64 * hi : 64 * hi + 64],
                            ALU.add,
                        )

                wo_tiles = {}

                def emit_wo_dma(co):
                    wt_ = wmvp.tile([128, 2 * TQ], BF16, tag=f"wmv{co}", name=f"wo{co}")
                    nc.scalar.dma_start(out=wt_[:], in_=woT[128 * co : 128 * co + 128, :])
                    wo_tiles[co] = wt_

                # ---------- eb slab prefetch (forward via negative stride) ----
                ebs = {}

                def emit_eb(hh):
                    eb_ = ebp.tile([128, W_EB], BF16, tag="eb", name=f"eb{hh}")
                    q = nc.sync if hh % 2 == 0 else nc.scalar
                    if EB_FWD:
                        src = _craft(
                            erb_dram[None, :],
                            [[1, 128], [-1, W_EB]],
                            hh * ERB_STRIDE + 1919,
                        )
                    else:
                        src = _craft(
                            erb_dram[None, :], [[1, 128], [1, W_EB]], hh * ERB_STRIDE
                        )
                    q.dma_start(out=eb_[:], in_=src)
                    ebs[hh] = eb_

                # ---------- normalization chain ----------
                norm_state = {}

                def emit_stage_evacA(p, opA):
                    onA = onp.tile([128, 2 * TQ], BF16, tag="onum", name=f"onA{p}")
                    nc.vector.tensor_copy(out=onA[0:65, :], in_=opA[0:65, :])
                    dd = drp2.tile([2 * 2 * TQ], BF16, tag="dend", name=f"dd{p}")
                    nc.gpsimd.dma_start(
                        out=_craft(dd[None, :], [[0, 1], [1, 2 * TQ]], 0),
                        in_=onA[64:65, :],
                    )
                    norm_state[p] = {"onA": onA, "dd": dd}

                def emit_stage_evacB(p):
                    st = norm_state[p]
                    opB = st.pop("opB")
                    onB = onp.tile([128, 2 * TQ], BF16, tag="onum", name=f"onB{p}")
                    nc.vector.tensor_copy(out=onB[64:128, :], in_=opB[64:128, :])
                    nc.scalar.activation(onB[0:1, :], opB[0:1, :], AF.Copy)
                    nc.gpsimd.dma_start(
                        out=_craft(st["dd"][None, :], [[0, 1], [1, 2 * TQ]], 2 * TQ),
                        in_=onB[0:1, :],
                    )
                    st["onB"] = onB

                def emit_norm_d16(p):
                    st = norm_state[p]
                    d16 = d16p.tile([128, 16], BF16, tag="d16", name=f"d16_{p}")
                    nc.gpsimd.dma_start(
                        out=d16[:],
                        in_=_craft(st["dd"][None, :], [[1, 128], [1024, 2], [128, 8]], 0),
                    )
                    st["d16"] = d16

                def emit_norm_recip(p):
                    st = norm_state[p]
                    r16 = d16p.tile([128, 16], BF16, tag="r16", name=f"r16_{p}")
                    with nc.allow_low_precision(reason="bf16 softmax denom ~0.4% ok"):
                        nc.vector.reciprocal(r16[:], st["d16"][:])
                    st["r16"] = r16

                def emit_norm_rdw(p):
                    st = norm_state[p]
                    rd = drp2.tile([2 * 2 * TQ], BF16, tag="recd", name=f"rd{p}")
                    nc.gpsimd.dma_start(
                        out=_craft(rd[None, :], [[1, 128], [1024, 2], [128, 8]], 0),
                        in_=st["r16"][:],
                    )
                    st["rd"] = rd

                def emit_norm_rp2(p):
                    st = norm_state[p]
                    rp2 = rp2p.tile([128, 2 * TQ], BF16, tag="rp2", name=f"rp2_{p}")
                    nc.gpsimd.dma_start(
                        out=rp2[0:64, :],
                        in_=_craft(st["rd"][None, :], [[0, 64], [1, 2 * TQ]], 0),
                    )
                    nc.gpsimd.dma_start(
                        out=rp2[64:128, :],
                        in_=_craft(st["rd"][None, :], [[0, 64], [1, 2 * TQ]], 2 * TQ),
                    )
                    st["rp2"] = rp2

                def emit_norm_final(p):
                    st = norm_state.pop(p)
                    nc.vector.tensor_tensor(
                        oT[0:64, p, :], st["onA"][0:64, :], st["rp2"][0:64, :], ALU.mult
                    )
                    nc.vector.tensor_tensor(
                        oT[64:128, p, :], st["onB"][64:128, :], st["rp2"][64:128, :],
                        ALU.mult,
                    )

                # ---------- attention pair ----------
                def emit_pair(p, hooks):
                    hA, hB = 2 * p, 2 * p + 1
                    ebA, ebB = ebs.pop(hA), ebs.pop(hB)
                    opA = opp.tile([128, 2 * TQ], F32, tag="OP", name=f"opA{p}")
                    ptAs = {}
                    ptBs = {}

                    def emit_pv_a(c):
                        pt_ = ptAs.pop(c)
                        for tqh in range(2):
                            nc.tensor.matmul(
                                opA[0:65, TQ * tqh : TQ * tqh + TQ],
                                vp[:, c, p, 0:65],
                                pt_[:, TQ * tqh : TQ * tqh + TQ],
                                start=(c == 0),
                                stop=(c == 7),
                            )

                    for c in range(8):
                        sps = []
                        for hp0 in (0, 64):
                            sp = spp.tile([128, 2 * TQ], F32, tag="S",
                                          name=f"s{2 * p + hp0 // 64}_{c}")
                            for tqh in range(2):
                                nc.tensor.matmul(
                                    sp[:, TQ * tqh : TQ * tqh + TQ],
                                    kT[hp0 : hp0 + 64, p, 128 * c : 128 * c + 128],
                                    qT[hp0 : hp0 + 64, p, TQ * tqh : TQ * tqh + TQ],
                                    start=True,
                                    stop=True,
                                )
                            sps.append(sp)
                        for fn in hooks.get(c, ()):
                            fn()
                        if EB_FWD:
                            sc = 896 - 128 * c
                        else:
                            s0 = 1023 + 128 * c
                        for hi, (sp, eb_) in enumerate(zip(sps, (ebA, ebB))):
                            pt0 = pt0p.tile([128, 2 * TQ], BF16, tag="pt0")
                            nc.scalar.activation(pt0[:], sp[:], AF.Exp, scale=0.125)
                            ptp_ = ptAp if hi == 0 else ptBp
                            pt_ = ptp_.tile([128, 2 * TQ], BF16,
                                            tag="ptA" if hi == 0 else "ptB",
                                            name=f"pt{2 * p + hi}_{c}")
                            ebv = (
                                eb_[:, sc : sc + 2 * TQ]
                                if EB_FWD
                                else eb_[:, s0 - (2 * TQ - 1) : s0 + 1][:, ::-1]
                            )
                            nc.vector.tensor_tensor(pt_[:], pt0[:], ebv, ALU.mult)
                            (ptAs if hi == 0 else ptBs)[c] = pt_
                        if c >= 1:
                            emit_pv_a(c - 1)
                    emit_pv_a(7)
                    emit_stage_evacA(p, opA)
                    # head B: dense burst into the same banks (M=64 @ base 64
                    # + concurrent col-tiled M=1 denominator @ partition 0)
                    opB = opp.tile([128, 2 * TQ], F32, tag="OP", name=f"opB{p}")
                    for c in range(8):
                        pt_ = ptBs.pop(c)
                        for tqh in range(2):
                            nc.tensor.matmul(
                                opB[64:128, TQ * tqh : TQ * tqh + TQ],
                                vp[:, c, p, 65:129],
                                pt_[:, TQ * tqh : TQ * tqh + TQ],
                                start=(c == 0),
                                stop=(c == 7),
                            )
                            nc.tensor.matmul(
                                opB[0:1, TQ * tqh : TQ * tqh + TQ],
                                ones8[:, 0:1],
                                pt_[:, TQ * tqh : TQ * tqh + TQ],
                                start=(c == 0),
                                stop=(c == 7),
                            )
                    norm_state[p]["opB"] = opB

                # ---------- schedule ----------
                def add_hook(hooks, c, fn):
                    hooks.setdefault(c, []).append(fn)

                # pre-attention ramp: q0,q1 as soon as xq lands; v0 after xv;
                # k0,k1 after xk.  S-tag tiles are free until pair 0.
                emit_q_mms(0, "S")
                emit_q_evac(0)
                emit_q_mms(1, "S")
                emit_q_evac(1)
                emit_v_job(0, "PJ")
                emit_w_dma("k", wkT, 1)
                emit_k_mms(0, "S")
                emit_k_evac(0)
                emit_k_mms(1, "S")
                emit_k_evac(1)
                for hh in range(4):  # eb slabs for pairs 0 and 1
                    emit_eb(hh)

                for p in range(8):
                    hooks = {}
                    if p + 1 < 8:
                        add_hook(hooks, 0, lambda p=p: emit_eb(2 * p + 2))
                        add_hook(hooks, 1, lambda p=p: emit_eb(2 * p + 3))
                    if p == 0:
                        for c, to in ((0, 1), (1, 2), (2, 3), (3, 4), (4, 5), (5, 6), (6, 7)):
                            add_hook(hooks, c, lambda to=to: emit_v_job(to, "PJ"))
                        add_hook(hooks, 2, lambda: emit_w_dma("q", wqT, 2))
                        add_hook(hooks, 5, lambda: emit_w_dma("k", wkT, 2))
                    else:
                        # norm chain for pair p-1 (B-side evac deferred here so
                        # the PV_B burst never stalls this pair's exp/mult)
                        add_hook(hooks, 1, lambda p=p: emit_stage_evacB(p - 1))
                        add_hook(hooks, 2, lambda p=p: emit_norm_d16(p - 1))
                        add_hook(hooks, 3, lambda p=p: emit_norm_recip(p - 1))
                        add_hook(hooks, 4, lambda p=p: emit_norm_rdw(p - 1))
                        add_hook(hooks, 5, lambda p=p: emit_norm_rp2(p - 1))
                        add_hook(hooks, 7, lambda p=p: emit_norm_final(p - 1))
                        # next q/k jobs: MMs and evacs at different chunks
                        if p + 1 < 8:
                            add_hook(hooks, 0, lambda p=p: emit_q_mms(p + 1, "PJ"))
                            add_hook(hooks, 3, lambda p=p: emit_q_evac(p + 1))
                            add_hook(hooks, 4, lambda p=p: emit_k_mms(p + 1, "PJ"))
                            add_hook(hooks, 7, lambda p=p: emit_k_evac(p + 1))
                        if p + 2 < 8:
                            add_hook(hooks, 2, lambda p=p: emit_w_dma("q", wqT, p + 2))
                            add_hook(hooks, 5, lambda p=p: emit_w_dma("k", wkT, p + 2))
                        if 1 <= p <= 4:
                            add_hook(hooks, 3, lambda p=p: emit_wo_dma(2 * p - 2))
                            add_hook(hooks, 6, lambda p=p: emit_wo_dma(2 * p - 1))
                    emit_pair(p, hooks)

                # ---------- tail: norm(7) + output projection ----------
                emit_stage_evacB(7)
                emit_norm_d16(7)
                emit_norm_recip(7)
                emit_norm_rdw(7)
                emit_norm_rp2(7)
                emit_norm_final(7)
                for to in range(8):
                    tag = "PJ" if to % 3 == 2 else "S"
                    pool = pjp if tag == "PJ" else spp
                    sp = pool.tile([128, 2 * TQ], F32, tag=tag, name=f"y{to}")
                    for fh in range(2):
                        for co in range(8):
                            nc.tensor.matmul(
                                sp[:, TQ * fh : TQ * fh + TQ],
                                oT[:, co, 128 * to : 128 * to + 128],
                                wo_tiles[co][:, TQ * fh : TQ * fh + TQ],
                                start=(co == 0),
                                stop=(co == 7),
                            )
                    yst = ystp.tile([128, 2 * TQ], F32, tag="yst")
                    nc.vector.tensor_tensor(yst[:], sp[:], borep[:], ALU.add)
                    q = nc.sync if to % 2 == 0 else nc.scalar
                    q.dma_start(out=y_out[128 * to : 128 * to + 128, :], in_=yst[:])

    _split_multi_waits(nc)
    return nc


_NC_CACHE = None


def _get_nc():
    global _NC_CACHE
    if _NC_CACHE is None:
        _NC_CACHE = _build()
    return _NC_CACHE


def _bf(x):
    return np.ascontiguousarray(np.asarray(x, np.float32).astype(ml_dtypes.bfloat16))


def _prepare_in_maps(
    query, key_, value, Wq, bq, Wk, bk, Wv, bv, Wo, bo, bias_table, offset
):
    query = np.asarray(query, np.float32)
    key_ = np.asarray(key_, np.float32)
    value = np.asarray(value, np.float32)
    shared = {
        "wqT": _bf(np.asarray(Wq, np.float32).T),
        "wkT": _bf(np.asarray(Wk, np.float32).T),
        "wvT": _bf(np.asarray(Wv, np.float32).T),
        "woT": _bf(np.asarray(Wo, np.float32).T),
        "bqr": _bf(np.asarray(bq, np.float32)),
        "bk2": np.ascontiguousarray(np.asarray(bk, np.float32).reshape(8, 128).T),
        "bv1": np.ascontiguousarray(np.asarray(bv, np.float32)),
        "bo1": np.ascontiguousarray(np.asarray(bo, np.float32)),
        "offs": np.ascontiguousarray(np.asarray(offset, np.float32)),
    }
    tab = np.asarray(bias_table, np.float32)  # [2T-1, H]
    padf = np.concatenate([tab[0:1], tab, tab[-1:]], axis=0)  # [2T+1, H] forward
    padfT = padf[::-1].T  # [H, 2T+1] reversed (both eb modes)
    # taps for the 128-partition blend: rt3[p, h*18 + i] = padfT[h, min(16p+i, 2T)]
    idx = np.minimum(np.arange(128)[:, None] * 16 + np.arange(18)[None, :], 2 * T)
    rt3 = padfT[:, idx]  # [H, 128, 18]
    rt3 = np.ascontiguousarray(rt3.transpose(1, 0, 2).reshape(128, 288))
    shared["rt3"] = rt3.astype(np.float32)

    in_maps = []
    for b in range(B):
        m = dict(shared)
        m["xqT"] = _bf(query[b].T)
        m["xkT"] = _bf(key_[b].T)
        m["xvT"] = _bf(value[b].T)
        in_maps.append(m)
    return in_maps


def kernel(**inputs):
    in_maps = _prepare_in_maps(
        inputs["query"], inputs["key_"], inputs["value"],
        inputs["Wq"], inputs["bq"], inputs["Wk"], inputs["bk"],
        inputs["Wv"], inputs["bv"], inputs["Wo"], inputs["bo"],
        inputs["bias_table"], inputs["offset"],
    )
    nc = _get_nc()
    res = run_bass_kernel_spmd(nc, in_maps, list(range(B)))
    out = np.stack([res.results[b]["y"] for b in range(B)], axis=0)
    return out.astype(np.float32)


# revision 22
# speedup vs baseline: 9.5689x; 1.0054x over previous
"""Trainium2 Bass kernel for CustomTemporalAttention.

B=8, T=1024, E=1024, H=16, D=64. Pure batch data-parallel across 8 cores.

v3: restructured schedule around dedicated PSUM roles so the PE never
blocks on the softmax pipeline and the HAM clock stays warm:

  - PSUM (8 banks): S ping-pong 2x[128,1024] (4) + projection slot
    1x[128,1024] (2) + PV accumulator 1x[128,1024] (2).
  - PV is head-SERIALIZED per pair: head A ([vA|1], M=65) accumulates
    c0..7 while head B's pt tiles are retained in SBUF; A is evacuated,
    then head B re-uses the same banks as a dense 16-MM burst with
    M=64 at base partition 64 plus a CONCURRENT col-tiled M=1 ones
    matmul at partition 0 for the denominator (HW-verified exact).
  - softmax denominators: one PSUM->SBUF copy carries num+den; den rows
    bounce through DRAM ([2048] -> [128,16]) for a partition-parallel
    reciprocal, broadcast back as stride-0 reads; final normalize is two
    bf16 DVE mults writing oT[0:64]/[64:128] directly (no gpsimd, no
    odd-head bounce).
  - exp-bias slabs are FORWARD-readable: the Toeplitz window DMA uses a
    negative partition stride (HW-verified exact), so the DVE multiply
    reads step +1 / 4B-aligned and packs 2 bf16 per cycle.
  - q-evac on ACT (Copy) with the bias folded in via a K=1 ones-row
    matmul; k-evac on DVE tensor_scalar (engine balance).  Projection
    MMs and their evacuations are emitted at different chunks so the
    strict per-engine FIFOs never stall the exp/mult pipeline.
  - inputs spread over sync+scalar HWDGE queues; weights + table bounce
    on the gpsimd SWDGE queue (first weight tiles queued AHEAD of the
    phase-0 chain); 50 warmup matmuls hold the HAM clock until real
    work lands.
"""

import sys

sys.path.insert(0, "/opt/trn_rl_repo")

import ml_dtypes
import numpy as np

import concourse.bass as bass
import concourse.mybir as mybir
import concourse.tile as tile
from concourse.bass_utils import run_bass_kernel_spmd

F32 = mybir.dt.float32
BF16 = mybir.dt.bfloat16
AF = mybir.ActivationFunctionType
ALU = mybir.AluOpType

B, T, E, H = 8, 1024, 1024, 16
D = E // H  # 64
TQ = 512
W_EB = 1920  # eb slab width
ERB_STRIDE = 2048  # per-head stride of blended-table DRAM rows
# Forward-readable slabs via innermost -1 stride DMA are functionally
# correct but the DMA degenerates to element-granularity descriptors
# (~17.7us engine time per slab, 11x kernel slowdown) - keep reversed
# DVE reads instead.
EB_FWD = False


def _split_multi_waits(nc):
    """This walrus build accepts at most one sync-wait per instruction; hoist
    extras onto same-engine NoOp carriers placed immediately before."""
    n = 0
    for f in nc.m.functions:
        for blk in f.blocks:
            out = []
            for inst in blk.instructions:
                si = inst.sync_info
                waits = list(si.on_wait) if si and si.on_wait else []
                if len(waits) > 1:
                    for w in waits[:-1]:
                        n += 1
                        nop = mybir.InstNoOp(name=f"{inst.name}-ws{n}", ins=[], outs=[])
                        nop.engine = inst.engine
                        nop.sync_info = mybir.SyncInfo(on_wait=[w], on_update=[])
                        out.append(nop)
                    inst.sync_info = mybir.SyncInfo(
                        on_wait=[waits[-1]], on_update=list(si.on_update or [])
                    )
                out.append(inst)
            blk.instructions = out
    return n


def _craft(ap, dims, offset=None):
    c = ap.copy()
    c.ap = ap.ap.__class__(dims)
    if offset is not None:
        c.offset = offset
    return c


def _build():
    nc = bass.Bass()

    xqT = nc.declare_dram_parameter("xqT", [E, T], BF16, isOutput=False)
    xkT = nc.declare_dram_parameter("xkT", [E, T], BF16, isOutput=False)
    xvT = nc.declare_dram_parameter("xvT", [E, T], BF16, isOutput=False)
    wqT = nc.declare_dram_parameter("wqT", [E, E], BF16, isOutput=False)
    wkT = nc.declare_dram_parameter("wkT", [E, E], BF16, isOutput=False)
    wvT = nc.declare_dram_parameter("wvT", [E, E], BF16, isOutput=False)
    woT = nc.declare_dram_parameter("woT", [E, E], BF16, isOutput=False)
    bqr = nc.declare_dram_parameter("bqr", [E], BF16, isOutput=False)
    bk2 = nc.declare_dram_parameter("bk2", [128, 8], F32, isOutput=False)
    bv1 = nc.declare_dram_parameter("bv1", [E], F32, isOutput=False)
    bo1 = nc.declare_dram_parameter("bo1", [E], F32, isOutput=False)
    rt3 = nc.declare_dram_parameter("rt3", [128, 288], F32, isOutput=False)
    offs = nc.declare_dram_parameter("offs", [1], F32, isOutput=False)
    y_out = nc.declare_dram_parameter("y", [T, E], F32, isOutput=True)

    with tile.TileContext(nc) as tc:
        with (
            tc.tile_pool(name="persist", bufs=1) as persist,
            tc.tile_pool(name="small", bufs=1) as small,
            tc.tile_pool(name="dram", bufs=1, space="DRAM") as drp,
        ):
            # persistent SBUF state
            xq = persist.tile([128, 8, T], BF16, tag="xq")
            xk = persist.tile([128, 8, T], BF16, tag="xk")
            xv = persist.tile([128, 8, T], BF16, tag="xv")
            qT = persist.tile([128, 8, T], BF16, tag="qT")
            kT = persist.tile([128, 8, T], BF16, tag="kT")
            # per-pair output tiles so outproj co-chunks depend only on their
            # own pair's normalize (single-tile oT serialized the whole
            # output projection behind the last pair's norm)
            oTs = [
                persist.tile([128, T], BF16, tag=f"oT{p}", name=f"oT{p}")
                for p in range(8)
            ]
            # per-pair value pages: [vA(0:64) | 1 | vB(65:129) | 1 | pad]
            vp = persist.tile([128, 8, 8, 132], BF16, tag="vp")
            bvrep = persist.tile([128, E], F32, tag="bvrep")
            borep = persist.tile([128, E], F32, tag="borep")
            bks = small.tile([128, 8], F32, tag="bks")
            bqrow = small.tile([1, E], BF16, tag="bqrow")
            ones8 = small.tile([128, 8], BF16, tag="ones8")
            ones_row = small.tile([1, TQ], BF16, tag="ones_row")
            nc.vector.memset(ones8[:], 1.0)
            nc.vector.memset(ones_row[:], 1.0)
            nc.vector.memset(vp[:, :, :, 64:65], 1.0)
            nc.vector.memset(vp[:, :, :, 129:130], 1.0)

            # small loads
            nc.sync.dma_start(out=bks[:], in_=bk2[:])
            nc.sync.dma_start(out=bvrep[:], in_=_craft(bv1[:], [[0, 128], [1, E]], 0))
            nc.sync.dma_start(out=borep[:], in_=_craft(bo1[:], [[0, 128], [1, E]], 0))
            nc.gpsimd.dma_start(out=bqrow[:], in_=bqr[None, :])

            with (
                tc.tile_pool(name="wt8", bufs=3) as wt8p,     # [128,8,128] q/k w chunks
                tc.tile_pool(name="wmv", bufs=1) as wmvp,     # [128,1024] wv then wo
                tc.tile_pool(name="eb", bufs=3) as ebp,       # [128,1920] exp-bias slabs
                tc.tile_pool(name="pt0", bufs=3) as pt0p,     # exp(S) pre-bias
                tc.tile_pool(name="ptA", bufs=3) as ptAp,     # P tiles head A
                tc.tile_pool(name="ptB", bufs=8) as ptBp,     # P tiles head B (full pair)
                tc.tile_pool(name="onum", bufs=2) as onp,     # staged num+den
                tc.tile_pool(name="rp2", bufs=2) as rp2p,     # bcast reciprocals
                tc.tile_pool(name="d16", bufs=2) as d16p,
                tc.tile_pool(name="yst", bufs=2) as ystp,
                tc.tile_pool(name="SP", bufs=2, space="PSUM") as spp,   # S ping-pong
                tc.tile_pool(name="PJ", bufs=1, space="PSUM") as pjp,   # projections
                tc.tile_pool(name="OP", bufs=1, space="PSUM") as opp,   # PV accum
                tc.tile_pool(name="dr2", bufs=2, space="DRAM") as drp2,
            ):
                # ---------- warmup: hold the PE HAM clock at 8/8 ----------
                wk_ps = pjp.tile([128, 2 * TQ], F32, tag="PJ", name="warm")
                for i in range(50):
                    nc.tensor.matmul(
                        wk_ps[0:8, 0:8], ones8[:], ones8[:], start=True, stop=True
                    )

                # ---------- first weight tiles AHEAD of phase-0 on gpsimd ----
                wt8_tiles = {}

                def emit_w_dma(name, w_in, fo):
                    wt8 = wt8p.tile([128, 8, 128], BF16, tag="wt8", name=f"w{name}{fo}")
                    nc.gpsimd.dma_start(
                        out=wt8[:],
                        in_=w_in[:, 128 * fo : 128 * fo + 128].rearrange(
                            "(e p) f -> p e f", p=128
                        ),
                    )
                    wt8_tiles[(name, fo)] = wt8

                emit_w_dma("q", wqT, 0)
                emit_w_dma("q", wqT, 1)
                emit_w_dma("k", wkT, 0)

                # ---------- phase 0: blended exp(bias) table (forward) ------
                p0ctx = tc.tile_pool(name="p0", bufs=1)
                p0 = p0ctx.__enter__()
                rt3s = p0.tile([128, 288], F32, tag="rt3s")
                nc.scalar.dma_start(out=rt3s[:], in_=rt3[:])
                off_sb = p0.tile([1, 1], F32, tag="off")
                nc.scalar.dma_start(out=off_sb[:], in_=offs[None, :])
                th = p0.tile([1, 1], F32, tag="th")
                nc.scalar.activation(th[:], off_sb[:], AF.Tanh)
                w8 = p0.tile([1, 1], F32, tag="w8")
                nc.vector.tensor_scalar_mul(w8[:], th[:], 4.0)  # 8*u = 4*tanh
                abc = p0.tile([1, 3], F32, tag="abc")
                nc.vector.tensor_scalar(abc[:, 0:1], w8[:], -1.0, 0.0, ALU.mult, ALU.max)
                nc.vector.tensor_scalar(abc[:, 2:3], w8[:], 1.0, 0.0, ALU.mult, ALU.max)
                tsum = p0.tile([1, 1], F32, tag="tsum")
                nc.vector.tensor_tensor(tsum[:], abc[:, 0:1], abc[:, 2:3], ALU.add)
                nc.vector.tensor_scalar(abc[:, 1:2], tsum[:], -1.0, 8.0, ALU.mult, ALU.add)
                abc_dram = drp.tile([3], F32, tag="abc_dram")
                nc.gpsimd.dma_start(out=abc_dram[None, :], in_=abc[:])
                abc128 = p0.tile([128, 3], F32, tag="abc128")
                nc.gpsimd.dma_start(
                    out=abc128[:], in_=_craft(abc_dram[:], [[0, 128], [1, 3]], 0)
                )

                # taps: rt3s viewed [128, 16h, 18]; blend 3 shifted slices
                rt3v = rt3s[:].rearrange("p (h i) -> p h i", i=18)
                rb3 = p0.tile([128, 16, 16], F32, tag="rb3")
                tmp3 = p0.tile([128, 16, 16], F32, tag="tmp3")
                a0, a2 = (2, 0)  # reversed-table blend weights (both eb modes)
                nc.vector.tensor_scalar(
                    rb3[:], rt3v[:, :, a0 : a0 + 16], abc128[:, 0:1], None, ALU.mult
                )
                nc.vector.tensor_scalar(
                    tmp3[:], rt3v[:, :, 1:17], abc128[:, 1:2], None, ALU.mult
                )
                nc.vector.tensor_tensor(rb3[:], rb3[:], tmp3[:], ALU.add)
                nc.vector.tensor_scalar(
                    tmp3[:], rt3v[:, :, a2 : a2 + 16], abc128[:, 2:3], None, ALU.mult
                )
                nc.vector.tensor_tensor(rb3[:], rb3[:], tmp3[:], ALU.add)
                erb3 = p0.tile([128, 16, 16], BF16, tag="erb3")
                nc.scalar.activation(erb3[:], rb3[:], AF.Exp, scale=0.125)
                erb_dram = drp.tile([H * ERB_STRIDE], BF16, tag="erb_dram")
                nc.gpsimd.dma_start(
                    out=_craft(
                        erb_dram[None, :], [[16, 128], [ERB_STRIDE, 16], [1, 16]], 0
                    ),
                    in_=erb3[:],
                )
                p0ctx.__exit__(None, None, None)

                # ---------- bulk input loads: spread across both HWDGE queues ----
                for eo in (0, 2, 4, 6):
                    nc.sync.dma_start(out=xq[:, eo, :], in_=xqT[128 * eo : 128 * eo + 128, :])
                for eo in (1, 3, 5, 7):
                    nc.scalar.dma_start(out=xq[:, eo, :], in_=xqT[128 * eo : 128 * eo + 128, :])
                for eo in (0, 2, 4, 6):
                    nc.sync.dma_start(out=xv[:, eo, :], in_=xvT[128 * eo : 128 * eo + 128, :])
                for eo in (1, 3, 5, 7):
                    nc.scalar.dma_start(out=xv[:, eo, :], in_=xvT[128 * eo : 128 * eo + 128, :])
                wv_tiles = {}
                for eo in range(8):
                    wt_ = wmvp.tile([128, 2 * TQ], BF16, tag=f"wmv{eo}", name=f"wv{eo}")
                    q = nc.sync if eo % 2 == 0 else nc.scalar
                    q.dma_start(out=wt_[:], in_=wvT[128 * eo : 128 * eo + 128, :])
                    wv_tiles[eo] = wt_
                for eo in (0, 2, 4, 6):
                    nc.sync.dma_start(out=xk[:, eo, :], in_=xkT[128 * eo : 128 * eo + 128, :])
                for eo in (1, 3, 5, 7):
                    nc.scalar.dma_start(out=xk[:, eo, :], in_=xkT[128 * eo : 128 * eo + 128, :])

                # ---------- projection jobs (MMs and evacs split) ----------
                pj_state = {}

                def emit_q_mms(fo, tag):
                    wt8 = wt8_tiles.pop(("q", fo))
                    pool = spp if tag == "S" else pjp
                    sp = pool.tile([128, 2 * TQ], F32, tag=tag, name=f"pq{fo}")
                    for tqh in range(2):
                        nc.tensor.matmul(
                            sp[:, TQ * tqh : TQ * tqh + TQ],
                            bqrow[0:1, 128 * fo : 128 * fo + 128],
                            ones_row[0:1, :],
                            start=True,
                            stop=False,
                        )
                        for eo in range(8):
                            nc.tensor.matmul(
                                sp[:, TQ * tqh : TQ * tqh + TQ],
                                wt8[:, eo, :],
                                xq[:, eo, TQ * tqh : TQ * tqh + TQ],
                                start=False,
                                stop=(eo == 7),
                            )
                    pj_state[("q", fo)] = sp

                def emit_q_evac(fo):
                    sp = pj_state.pop(("q", fo))
                    nc.scalar.activation(qT[:, fo, :], sp[:], AF.Copy)

                def emit_k_mms(fo, tag):
                    wt8 = wt8_tiles.pop(("k", fo))
                    pool = spp if tag == "S" else pjp
                    sp = pool.tile([128, 2 * TQ], F32, tag=tag, name=f"pk{fo}")
                    for tqh in range(2):
                        for eo in range(8):
                            nc.tensor.matmul(
                                sp[:, TQ * tqh : TQ * tqh + TQ],
                                wt8[:, eo, :],
                                xk[:, eo, TQ * tqh : TQ * tqh + TQ],
                                start=(eo == 0),
                                stop=(eo == 7),
                            )
                    pj_state[("k", fo)] = sp

                def emit_k_evac(fo):
                    sp = pj_state.pop(("k", fo))
                    nc.vector.tensor_scalar(
                        kT[:, fo, :], sp[:], 1.0, bks[:, fo : fo + 1], ALU.mult, ALU.add
                    )

                def emit_v_job(to, tag):
                    pool = spp if tag == "S" else pjp
                    sp = pool.tile([128, 2 * TQ], F32, tag=tag, name=f"pv{to}")
                    for fv in range(2):
                        for eo in range(8):
                            nc.tensor.matmul(
                                sp[:, TQ * fv : TQ * fv + TQ],
                                xv[:, eo, 128 * to : 128 * to + 128],
                                wv_tiles[eo][:, TQ * fv : TQ * fv + TQ],
                                start=(eo == 0),
                                stop=(eo == 7),
                            )
                    # scatter into [vA |1| vB |1| pad] pages (+ bias)
                    spv = sp[:].rearrange("p (pr x) -> p pr x", x=128)
                    bvv = bvrep[:].rearrange("p (pr x) -> p pr x", x=128)
                    for hi in range(2):
                        nc.vector.tensor_tensor(
                            vp[:, to, :, 65 * hi : 65 * hi + 64],
                            spv[:, :, 64 * hi : 64 * hi + 64],
                            bvv[:, :, 64 * hi : 64 * hi + 64],
                            ALU.add,
                        )

                wo_tiles = {}

                def emit_wo_dma(co):
                    wt_ = wmvp.tile([128, 2 * TQ], BF16, tag=f"wmv{co}", name=f"wo{co}")
                    nc.gpsimd.dma_start(out=wt_[:], in_=woT[128 * co : 128 * co + 128, :])
                    wo_tiles[co] = wt_

                # ---------- eb slab prefetch (forward via negative stride) ----
                ebs = {}

                def emit_eb(hh):
                    eb_ = ebp.tile([128, W_EB], BF16, tag="eb", name=f"eb{hh}")
                    # first four slabs split across both HWDGE queues (startup);
                    # in-loop slabs go on sync only -- the scalar engine must
                    # stay pure-compute during attention
                    q = nc.scalar if hh in (1, 3) else nc.sync
                    if EB_FWD:
                        src = _craft(
                            erb_dram[None, :],
                            [[1, 128], [-1, W_EB]],
                            hh * ERB_STRIDE + 1919,
                        )
                    else:
                        src = _craft(
                            erb_dram[None, :], [[1, 128], [1, W_EB]], hh * ERB_STRIDE
                        )
                    q.dma_start(out=eb_[:], in_=src)
                    ebs[hh] = eb_

                # ---------- normalization chain ----------
                norm_state = {}

                def emit_stage_evacA(p, opA):
                    onA = onp.tile([128, 2 * TQ], BF16, tag="onum", name=f"onA{p}")
                    nc.vector.tensor_copy(out=onA[0:65, :], in_=opA[0:65, :])
                    dd = drp2.tile([2 * 2 * TQ], BF16, tag="dend", name=f"dd{p}")
                    nc.gpsimd.dma_start(
                        out=_craft(dd[None, :], [[0, 1], [1, 2 * TQ]], 0),
                        in_=onA[64:65, :],
                    )
                    norm_state[p] = {"onA": onA, "dd": dd}

                def emit_stage_evacB(p):
                    st = norm_state[p]
                    opB = st.pop("opB")
                    onB = onp.tile([128, 2 * TQ], BF16, tag="onum", name=f"onB{p}")
                    nc.vector.tensor_copy(out=onB[64:128, :], in_=opB[64:128, :])
                    nc.scalar.activation(onB[0:1, :], opB[0:1, :], AF.Copy)
                    nc.gpsimd.dma_start(
                        out=_craft(st["dd"][None, :], [[0, 1], [1, 2 * TQ]], 2 * TQ),
                        in_=onB[0:1, :],
                    )
                    st["onB"] = onB

                def emit_norm_d16(p):
                    st = norm_state[p]
                    d16 = d16p.tile([128, 16], BF16, tag="d16", name=f"d16_{p}")
                    nc.gpsimd.dma_start(
                        out=d16[:],
                        in_=_craft(st["dd"][None, :], [[1, 128], [1024, 2], [128, 8]], 0),
                    )
                    st["d16"] = d16

                def emit_norm_recip(p):
                    st = norm_state[p]
                    r16 = d16p.tile([128, 16], BF16, tag="r16", name=f"r16_{p}")
                    with nc.allow_low_precision(reason="bf16 softmax denom ~0.4% ok"):
                        nc.vector.reciprocal(r16[:], st["d16"][:])
                    st["r16"] = r16

                def emit_norm_rdw(p):
                    st = norm_state[p]
                    rd = drp2.tile([2 * 2 * TQ], BF16, tag="recd", name=f"rd{p}")
                    nc.gpsimd.dma_start(
                        out=_craft(rd[None, :], [[1, 128], [1024, 2], [128, 8]], 0),
                        in_=st["r16"][:],
                    )
                    st["rd"] = rd

                def emit_norm_rp2(p):
                    st = norm_state[p]
                    rp2 = rp2p.tile([128, 2 * TQ], BF16, tag="rp2", name=f"rp2_{p}")
                    nc.gpsimd.dma_start(
                        out=rp2[0:64, :],
                        in_=_craft(st["rd"][None, :], [[0, 64], [1, 2 * TQ]], 0),
                    )
                    nc.gpsimd.dma_start(
                        out=rp2[64:128, :],
                        in_=_craft(st["rd"][None, :], [[0, 64], [1, 2 * TQ]], 2 * TQ),
                    )
                    st["rp2"] = rp2

                def emit_norm_final(p):
                    st = norm_state.pop(p)
                    nc.vector.tensor_tensor(
                        oTs[p][0:64, :], st["onA"][0:64, :], st["rp2"][0:64, :],
                        ALU.mult,
                    )
                    nc.vector.tensor_tensor(
                        oTs[p][64:128, :], st["onB"][64:128, :], st["rp2"][64:128, :],
                        ALU.mult,
                    )

                # ---------- attention pair ----------
                pvb_pend = {}

                def emit_pv_b(p, cs):
                    """Head B PV chunk group: M=64 @ base 64 + concurrent
                    col-tiled M=1 denominator @ partition 0."""
                    st = pvb_pend[p]
                    if "opB" not in st:
                        st["opB"] = opp.tile([128, 2 * TQ], F32, tag="OP",
                                             name=f"opB{p}")
                    opB = st["opB"]
                    for c in cs:
                        pt_ = st["ptBs"].pop(c)
                        for tqh in range(2):
                            nc.tensor.matmul(
                                opB[64:128, TQ * tqh : TQ * tqh + TQ],
                                vp[:, c, p, 65:129],
                                pt_[:, TQ * tqh : TQ * tqh + TQ],
                                start=(c == 0),
                                stop=(c == 7),
                            )
                            nc.tensor.matmul(
                                opB[0:1, TQ * tqh : TQ * tqh + TQ],
                                ones8[:, 0:1],
                                pt_[:, TQ * tqh : TQ * tqh + TQ],
                                start=(c == 0),
                                stop=(c == 7),
                            )
                    if cs[-1] == 7:
                        pvb_pend.pop(p)
                        norm_state[p]["opB"] = opB

                def emit_pair(p, hooks):
                    hA, hB = 2 * p, 2 * p + 1
                    ebA, ebB = ebs.pop(hA), ebs.pop(hB)
                    # opA allocated lazily at its first write so the OP pool
                    # rotation interleaves correctly with opB(p-1)'s alloc in
                    # this pair's c0 hook
                    opAbox = {}
                    ptAs = {}
                    ptBs = {}

                    def emit_pv_a(c):
                        if "t" not in opAbox:
                            opAbox["t"] = opp.tile([128, 2 * TQ], F32, tag="OP",
                                                   name=f"opA{p}")
                        opA = opAbox["t"]
                        pt_ = ptAs.pop(c)
                        for tqh in range(2):
                            nc.tensor.matmul(
                                opA[0:65, TQ * tqh : TQ * tqh + TQ],
                                vp[:, c, p, 0:65],
                                pt_[:, TQ * tqh : TQ * tqh + TQ],
                                start=(c == 0),
                                stop=(c == 7),
                            )

                    for c in range(8):
                        sps = []
                        for hp0 in (0, 64):
                            sp = spp.tile([128, 2 * TQ], F32, tag="S",
                                          name=f"s{2 * p + hp0 // 64}_{c}")
                            for tqh in range(2):
                                nc.tensor.matmul(
                                    sp[:, TQ * tqh : TQ * tqh + TQ],
                                    kT[hp0 : hp0 + 64, p, 128 * c : 128 * c + 128],
                                    qT[hp0 : hp0 + 64, p, TQ * tqh : TQ * tqh + TQ],
                                    start=True,
                                    stop=True,
                                )
                            sps.append(sp)
                        for fn in hooks.get(c, ()):
                            fn()
                        if EB_FWD:
                            sc = 896 - 128 * c
                        else:
                            s0 = 1023 + 128 * c
                        for hi, (sp, eb_) in enumerate(zip(sps, (ebA, ebB))):
                            pt0 = pt0p.tile([128, 2 * TQ], BF16, tag="pt0")
                            nc.scalar.activation(pt0[:], sp[:], AF.Exp, scale=0.125)
                            ptp_ = ptAp if hi == 0 else ptBp
                            pt_ = ptp_.tile([128, 2 * TQ], BF16,
                                            tag="ptA" if hi == 0 else "ptB",
                                            name=f"pt{2 * p + hi}_{c}")
                            ebv = (
                                eb_[:, sc : sc + 2 * TQ]
                                if EB_FWD
                                else eb_[:, s0 - (2 * TQ - 1) : s0 + 1][:, ::-1]
                            )
                            nc.vector.tensor_tensor(pt_[:], pt0[:], ebv, ALU.mult)
                            (ptAs if hi == 0 else ptBs)[c] = pt_
                        if c >= 2:
                            emit_pv_a(c - 2)
                    emit_pv_a(6)
                    emit_pv_a(7)
                    emit_stage_evacA(p, opAbox["t"])
                    pvb_pend[p] = {"ptBs": ptBs}

                # ---------- schedule ----------
                def add_hook(hooks, c, fn):
                    hooks.setdefault(c, []).append(fn)

                # pre-attention ramp: q0,q1 as soon as xq lands; v0 after xv;
                # k0,k1 after xk.  S-tag tiles are free until pair 0.
                emit_q_mms(0, "S")
                emit_q_evac(0)
                emit_q_mms(1, "S")
                emit_q_evac(1)
                emit_v_job(0, "PJ")
                emit_w_dma("k", wkT, 1)
                emit_k_mms(0, "S")
                emit_k_evac(0)
                emit_k_mms(1, "S")
                emit_k_evac(1)
                for hh in range(4):  # eb slabs for pairs 0 and 1
                    emit_eb(hh)

                for p in range(8):
                    hooks = {}
                    if p + 1 < 8:
                        add_hook(hooks, 0, lambda p=p: emit_eb(2 * p + 2))
                        add_hook(hooks, 1, lambda p=p: emit_eb(2 * p + 3))
                    if p == 0:
                        for c, to in ((0, 1), (1, 2), (2, 3), (3, 4), (4, 5), (5, 6), (6, 7)):
                            add_hook(hooks, c, lambda to=to: emit_v_job(to, "PJ"))
                        add_hook(hooks, 2, lambda: emit_w_dma("q", wqT, 2))
                        add_hook(hooks, 5, lambda: emit_w_dma("k", wkT, 2))
                    else:
                        # head-B PV of the previous pair, split across the
                        # first two chunks (fills the exp-wait slots instead
                        # of starving ACT/DVE at the boundary)
                        add_hook(hooks, 0, lambda p=p: emit_pv_b(p - 1, (0, 1, 2, 3)))
                        add_hook(hooks, 1, lambda p=p: emit_pv_b(p - 1, (4, 5, 6, 7)))
                        if p >= 2:
                            add_hook(hooks, 1, lambda p=p: emit_norm_final(p - 2))
                        # norm chain for pair p-1
                        add_hook(hooks, 2, lambda p=p: emit_stage_evacB(p - 1))
                        add_hook(hooks, 3, lambda p=p: emit_norm_d16(p - 1))
                        add_hook(hooks, 4, lambda p=p: emit_norm_recip(p - 1))
                        add_hook(hooks, 5, lambda p=p: emit_norm_rdw(p - 1))
                        add_hook(hooks, 6, lambda p=p: emit_norm_rp2(p - 1))
                        # next q/k jobs: MMs and evacs at different chunks
                        if p + 1 < 8:
                            add_hook(hooks, 2, lambda p=p: emit_q_mms(p + 1, "PJ"))
                            add_hook(hooks, 4, lambda p=p: emit_q_evac(p + 1))
                            add_hook(hooks, 4, lambda p=p: emit_k_mms(p + 1, "PJ"))
                            add_hook(hooks, 6, lambda p=p: emit_k_evac(p + 1))
                        if p + 2 < 8:
                            add_hook(hooks, 3, lambda p=p: emit_w_dma("q", wqT, p + 2))
                            add_hook(hooks, 5, lambda p=p: emit_w_dma("k", wkT, p + 2))
                        if 1 <= p <= 4:
                            add_hook(hooks, 3, lambda p=p: emit_wo_dma(2 * p - 2))
                            add_hook(hooks, 6, lambda p=p: emit_wo_dma(2 * p - 1))
                    emit_pair(p, hooks)

                # ---------- tail: PV_B(7) + norm(6,7) + output projection ----
                emit_pv_b(7, (0, 1, 2, 3))
                emit_pv_b(7, (4, 5, 6, 7))
                emit_norm_final(6)
                emit_stage_evacB(7)
                emit_norm_d16(7)
                emit_norm_recip(7)
                emit_norm_rdw(7)
                emit_norm_rp2(7)
                emit_norm_final(7)
                for to in range(8):
                    tag = "PJ" if to % 3 == 2 else "S"
                    pool = pjp if tag == "PJ" else spp
                    sp = pool.tile([128, 2 * TQ], F32, tag=tag, name=f"y{to}")
                    for fh in range(2):
                        for co in range(8):
                            nc.tensor.matmul(
                                sp[:, TQ * fh : TQ * fh + TQ],
                                oTs[co][:, 128 * to : 128 * to + 128],
                                wo_tiles[co][:, TQ * fh : TQ * fh + TQ],
                                start=(co == 0),
                                stop=(co == 7),
                            )
                    yst = ystp.tile([128, 2 * TQ], F32, tag="yst")
                    nc.vector.tensor_tensor(yst[:], sp[:], borep[:], ALU.add)
                    q = nc.sync if to % 2 == 0 else nc.scalar
                    q.dma_start(out=y_out[128 * to : 128 * to + 128, :], in_=yst[:])

    _split_multi_waits(nc)
    return nc


_NC_CACHE = None


def _get_nc():
    global _NC_CACHE
    if _NC_CACHE is None:
        _NC_CACHE = _build()
    return _NC_CACHE


def _bf(x):
    return np.ascontiguousarray(np.asarray(x, np.float32).astype(ml_dtypes.bfloat16))


def _prepare_in_maps(
    query, key_, value, Wq, bq, Wk, bk, Wv, bv, Wo, bo, bias_table, offset
):
    query = np.asarray(query, np.float32)
    key_ = np.asarray(key_, np.float32)
    value = np.asarray(value, np.float32)
    shared = {
        "wqT": _bf(np.asarray(Wq, np.float32).T),
        "wkT": _bf(np.asarray(Wk, np.float32).T),
        "wvT": _bf(np.asarray(Wv, np.float32).T),
        "woT": _bf(np.asarray(Wo, np.float32).T),
        "bqr": _bf(np.asarray(bq, np.float32)),
        "bk2": np.ascontiguousarray(np.asarray(bk, np.float32).reshape(8, 128).T),
        "bv1": np.ascontiguousarray(np.asarray(bv, np.float32)),
        "bo1": np.ascontiguousarray(np.asarray(bo, np.float32)),
        "offs": np.ascontiguousarray(np.asarray(offset, np.float32)),
    }
    tab = np.asarray(bias_table, np.float32)  # [2T-1, H]
    padf = np.concatenate([tab[0:1], tab, tab[-1:]], axis=0)  # [2T+1, H] forward
    padfT = padf[::-1].T  # [H, 2T+1] reversed (both eb modes)
    # taps for the 128-partition blend: rt3[p, h*18 + i] = padfT[h, min(16p+i, 2T)]
    idx = np.minimum(np.arange(128)[:, None] * 16 + np.arange(18)[None, :], 2 * T)
    rt3 = padfT[:, idx]  # [H, 128, 18]
    rt3 = np.ascontiguousarray(rt3.transpose(1, 0, 2).reshape(128, 288))
    shared["rt3"] = rt3.astype(np.float32)

    in_maps = []
    for b in range(B):
        m = dict(shared)
        m["xqT"] = _bf(query[b].T)
        m["xkT"] = _bf(key_[b].T)
        m["xvT"] = _bf(value[b].T)
        in_maps.append(m)
    return in_maps


def kernel(**inputs):
    in_maps = _prepare_in_maps(
        inputs["query"], inputs["key_"], inputs["value"],
        inputs["Wq"], inputs["bq"], inputs["Wk"], inputs["bk"],
        inputs["Wv"], inputs["bv"], inputs["Wo"], inputs["bo"],
        inputs["bias_table"], inputs["offset"],
    )
    nc = _get_nc()
    res = run_bass_kernel_spmd(nc, in_maps, list(range(B)))
    out = np.stack([res.results[b]["y"] for b in range(B)], axis=0)
    return out.astype(np.float32)
